# revision 1
# baseline (speedup 1.0000x reference)
"""Dense transformer block on 8 trn2 cores: K/V AllGather + token sharding.

Each core computes rmsnorm1 stats over all T tokens and K/V for ITS OWN GQA
group (tensor-parallel, no redundancy), publishes them with one small
AllGather (1 MB in / 8 MB out, bf16) that overlaps with the Q/score-scaling
compute for its own 256-token slice, then runs attention (all 32 heads),
proj, rmsnorm2 and the FULL-ffn SwiGLU MLP (11008 = 86*128) for its own
tokens only.  The single collective is hidden behind compute; each core
writes its own token slice of the output with no further sync.

Perf notes (guided by concourse TimelineSim, ~1.79 ms/core simulated):
- rmsnorm scaling commutes with the C-contraction (rstd varies only along
  tokens, the free axis), so Q/K/V/scale matmul RAW bf16 x and rstd is
  folded into the rope cos/sin tables, one V output mul, or the pre-relu
  scale mul.  x is fed bf16 from the host and read ONCE per token chunk.
- attention scores run 2 k-tiles ahead of the yP/csP accumulators so the PE
  never waits on the ACT-engine exp; the rmsnorm2 variance matmul trails the
  producing c-tile by one for the same reason.
- DMA issue is split across both HWDGE queues (SP + ACT), with each phase's
  first loads on the queue that is idle during the PRECEDING phase, and the
  first 4 proj-weight tiles prefetched during attention into a long-lived
  pool so phase C is not gated on attention's pools releasing their bytes.
- sequencer DMA-issue time scales with descriptor count, so the x stream
  uses batched 3D-pattern DMAs (4 per chunk instead of 32) and xT_own loads
  once into a persistent tile shared by the A2 variance pass and the phase-C
  residual add.
"""

import numpy as np
import ml_dtypes

import concourse.mybir as mybir
import concourse.tile as tile
from concourse import bacc
from concourse.bass_utils import run_bass_kernel_spmd

BF = ml_dtypes.bfloat16
F32 = mybir.dt.float32
BF16 = mybir.dt.bfloat16
AF = mybir.ActivationFunctionType

N_CORES = 8
T = 2048
C = 4096
HS = 128
NHF = 32          # full q heads
NG = 8            # kv groups
CT = C // 128     # 32 c-tiles
TCH = 512         # token chunk, phase A full-T pass
NCH = T // TCH
KT = T // 128     # 16 key tiles
FT = 86           # full ffn tiles (86*128 = 11008)
TO = T // N_CORES  # 256 own tokens
EPS = 1e-5
SF = 1.0 / float(np.sqrt(HS))

_CACHE = {}


def _build(cc=True):
    nc = bacc.Bacc(None, target_bir_lowering=False, num_devices=N_CORES)

    xT = nc.dram_tensor("xT", [C, T], BF16, kind="ExternalInput")
    xT_own = nc.dram_tensor("xT_own", [C, TO], F32, kind="ExternalInput")
    cosT = nc.dram_tensor("cosT", [128, T], F32, kind="ExternalInput")
    sinT = nc.dram_tensor("sinT", [128, T], F32, kind="ExternalInput")
    cos_own = nc.dram_tensor("cos_own", [128, TO], F32, kind="ExternalInput")
    sin_own = nc.dram_tensor("sin_own", [128, TO], F32, kind="ExternalInput")
    # K then V tile for this core's own group, w1-folded
    wKV = nc.dram_tensor("wKV", [2, 128, C], BF16, kind="ExternalInput")
    # all 32 q head tiles then all 32 scale tiles, w1-folded
    wQS = nc.dram_tensor("wQS", [2 * NHF, 128, C], BF16, kind="ExternalInput")
    scale_b = nc.dram_tensor("scale_b", [128, NHF], F32, kind="ExternalInput")
    projT = nc.dram_tensor("projT", [128, NHF, C], BF16, kind="ExternalInput")
    gate_w = nc.dram_tensor("gate_w", [FT, 128, C], BF16, kind="ExternalInput")
    up_w = nc.dram_tensor("up_w", [FT, 128, C], BF16, kind="ExternalInput")
    down_w = nc.dram_tensor("down_w", [CT, 128, FT * 128], BF16, kind="ExternalInput")
    outT = nc.dram_tensor("outT", [C, TO], F32, kind="ExternalOutput")

    with tile.TileContext(nc) as tc:
        with (
            tc.tile_pool(name="persist", bufs=1) as pp,
            tc.tile_pool(name="pdx", bufs=1) as pdx,
            tc.tile_pool(name="dram", bufs=1, space="DRAM") as dram,
        ):
            inv_c = pp.tile([128, 128], BF16, name="inv_c")
            nc.vector.memset(inv_c[:], 1.0 / C)
            mean_sf = pp.tile([128, 128], BF16, name="mean_sf")
            nc.vector.memset(mean_sf[:], SF / HS)
            ones128 = pp.tile([128, 128], BF16, name="ones128")
            nc.vector.memset(ones128[:], 1.0)
            eps_sb = pp.tile([128, 1], F32, name="eps_sb")
            nc.vector.memset(eps_sb[:], EPS)

            kv_in = dram.tile([2, 128, T], BF16, name="kv_in")
            kv_out = dram.tile([2 * NG, 128, T], BF16, name="kv_out",
                               addr_space="Shared")
            x1T = pdx.tile([128, CT, TO], BF16, name="x1T")
            n2T = pdx.tile([128, CT, TO], BF16, name="n2T")
            xow = pdx.tile([128, CT, TO], F32, name="xow")
            nc.scalar.dma_start(
                xow[:], xT_own.rearrange("(ct p) t -> p ct t", p=128))

            with tc.tile_pool(name="pqkv", bufs=1) as pqkv:

                # ---- Phase A1: full-T rmsnorm1 -> K (rope) and V ----------
                with (
                    tc.tile_pool(name="pa", bufs=1) as pa,
                    tc.tile_pool(name="pa3", bufs=3) as pa3,
                    tc.tile_pool(name="psA", bufs=1, space="PSUM") as psA,
                ):
                    # K/V weights are chunk-invariant: load once
                    kwt = pa.tile([128, CT, 128], BF16, name="kwt")
                    nc.scalar.dma_start(
                        kwt[:], wKV[0].rearrange("p (ct j) -> p ct j", j=128))
                    vwt = pa.tile([128, CT, 128], BF16, name="vwt")
                    nc.scalar.dma_start(
                        vwt[:], wKV[1].rearrange("p (ct j) -> p ct j", j=128))
                    # rmsnorm1 scaling commutes with the C-contraction
                    # (rstd varies only along tokens, the free axis), so K/V
                    # matmul RAW bf16 x and rstd is folded into the rope
                    # cos/sin (K) or one output mul (V).  x is read ONCE.
                    for ci in range(NCH):
                        t0 = ci * TCH
                        tsl = slice(t0, t0 + TCH)
                        xbf = pa.tile([128, CT, TCH], BF16, name=f"xbf_{ci}", tag="xbf", bufs=2)
                        cos_sb = pa3.tile([128, TCH], F32, name=f"cos_{ci}", tag="cos", bufs=2)
                        nc.scalar.dma_start(cos_sb[:], cosT[:, tsl])
                        sin_sb = pa3.tile([128, TCH], F32, name=f"sin_{ci}", tag="sin", bufs=2)
                        nc.scalar.dma_start(sin_sb[:], sinT[:, tsl])
                        varP = psA.tile([128, TCH], F32, name=f"varP_{ci}", tag="varP", bufs=2)
                        for cq in range(4):
                            nc.sync.dma_start(
                                xbf[:, cq * 8:(cq + 1) * 8, :],
                                xT[cq * 1024:(cq + 1) * 1024, tsl].rearrange(
                                    "(ct p) t -> p ct t", p=128))
                        for ct in range(CT):
                            sq = pa3.tile([128, TCH], BF16, name=f"sq_{ci}_{ct}", tag="sq", bufs=3)
                            nc.vector.tensor_mul(sq[:], xbf[:, ct, :], xbf[:, ct, :])
                            nc.tensor.matmul(
                                varP[:], inv_c[:], sq[:],
                                start=(ct == 0), stop=(ct == CT - 1),
                            )
                        sdv = pa3.tile([128, TCH], F32, name=f"sdv_{ci}", tag="sdv", bufs=2)
                        nc.scalar.activation(sdv[:], varP[:], AF.Sqrt, bias=eps_sb[:, 0:1])
                        rstd = pa3.tile([128, TCH], F32, name=f"rstd_{ci}", tag="rstd", bufs=2)
                        nc.vector.reciprocal(rstd[:], sdv[:])

                        cosr = pa3.tile([128, TCH], F32, name=f"cosr_{ci}", tag="cosr", bufs=2)
                        nc.vector.tensor_mul(cosr[:], cos_sb[:], rstd[:])
                        sinr = pa3.tile([128, TCH], F32, name=f"sinr_{ci}", tag="sinr", bufs=2)
                        nc.vector.tensor_mul(sinr[:], sin_sb[:], rstd[:])

                        # K for own group, rope with rstd-folded cos/sin
                        kP = psA.tile([128, TCH], F32, name=f"kP_{ci}", tag="kP", bufs=3)
                        for ct in range(CT):
                            nc.tensor.matmul(
                                kP[:], kwt[:, ct, :], xbf[:, ct, :],
                                start=(ct == 0), stop=(ct == CT - 1),
                            )
                        # V for own group: feature-major matmul, rstd fold,
                        # then DMA-transpose to token-major
                        vP = psA.tile([128, TCH], F32, name=f"vP_{ci}", tag="vP", bufs=2)
                        for ct in range(CT):
                            nc.tensor.matmul(
                                vP[:], vwt[:, ct, :], xbf[:, ct, :],
                                start=(ct == 0), stop=(ct == CT - 1),
                            )
                        raw = pa3.tile([128, TCH], F32, name=f"raw_{ci}", tag="raw", bufs=2)
                        nc.scalar.activation(raw[:], kP[:], AF.Copy)
                        rot = pa3.tile([128, TCH], F32, name=f"rot_{ci}", tag="rot", bufs=2)
                        nc.sync.dma_start(rot[0:64, :], raw[64:128, :])
                        nc.sync.dma_start(rot[64:128, :], raw[0:64, :])
                        t1 = pa3.tile([128, TCH], F32, name=f"t1_{ci}", tag="t1", bufs=2)
                        nc.vector.tensor_mul(t1[:], raw[:], cosr[:])
                        t2 = pa3.tile([128, TCH], F32, name=f"t2_{ci}", tag="t2", bufs=2)
                        nc.vector.tensor_mul(t2[:], rot[:], sinr[:])
                        ko = pa3.tile([128, TCH], BF16, name=f"ko_{ci}", tag="ko", bufs=2)
                        nc.vector.tensor_add(ko[:], t1[:], t2[:])
                        nc.scalar.dma_start(kv_in[0, :, tsl], ko[:])

                        vsc = pa3.tile([128, TCH], BF16, name=f"vsc_{ci}", tag="vsc", bufs=2)
                        nc.vector.tensor_mul(vsc[:], vP[:], rstd[:])
                        for tt in range(TCH // 128):
                            vtk = pa3.tile([128, 128], BF16, name=f"vtk_{ci}_{tt}", tag="vtk", bufs=2)
                            nc.scalar.dma_start(
                                vtk[:], vsc[:, tt * 128:(tt + 1) * 128], transpose=True)
                            nc.scalar.dma_start(
                                kv_in[1, :, (ci * (TCH // 128) + tt) * 128:
                                      (ci * (TCH // 128) + tt + 1) * 128],
                                vtk[:])

                    if cc:
                        nc.gpsimd.collective_compute(
                            "AllGather", mybir.AluOpType.bypass,
                            replica_groups=[list(range(N_CORES))],
                            ins=[kv_in.opt()], outs=[kv_out.opt()],
                        )

                # Q/yT/proj-prefetch live from A2 through C but NOT in A1:
                # allocating them after A1's pools close frees 64KB there so
                # the A1 chunk pipeline can double-buffer
                with tc.tile_pool(name="ppre", bufs=1) as ppre:
                    Q_sb = ppre.tile([128, NHF, TO], BF16, name="Q_sb")
                    yT_sb = ppre.tile([128, NHF, TO], BF16, name="yT_sb")
                    proj_pre = ppre.tile([128, 4, NHF, 128], BF16, name="proj_pre")
                    for pc in range(4):
                        nc.sync.dma_start(
                            proj_pre[:, pc], projT[:, :, pc * 128:(pc + 1) * 128])
                    # ---- Phase A2: own-token rmsnorm1 -> Q (rope) + scaling ---
                    with (
                        tc.tile_pool(name="pq", bufs=1) as pq,
                        tc.tile_pool(name="pq2", bufs=2) as pq2,
                        tc.tile_pool(name="pq3", bufs=3) as pq3,
                        tc.tile_pool(name="psQ", bufs=1, space="PSUM") as psQ,
                    ):
                        scbc = pq.tile([128, NHF, TO], BF16, name="scbc")
                        sb_sb = pq.tile([128, NHF], F32, name="sb_sb")
                        nc.sync.dma_start(sb_sb[:], scale_b[:])
                        co_sb = pq.tile([128, TO], F32, name="co_sb")
                        nc.sync.dma_start(co_sb[:], cos_own[:])
                        so_sb = pq.tile([128, TO], F32, name="so_sb")
                        nc.sync.dma_start(so_sb[:], sin_own[:])

                        varQ = psQ.tile([128, TO], F32, name="varQ")
                        xbo = pq.tile([128, CT, TO], BF16, name="xbo")
                        for ct in range(CT):
                            sq = pq3.tile([128, TO], BF16, name=f"sqq_{ct}", tag="sqq")
                            nc.vector.tensor_mul(sq[:], xow[:, ct, :], xow[:, ct, :])
                            nc.vector.tensor_copy(xbo[:, ct, :], xow[:, ct, :])
                            nc.tensor.matmul(
                                varQ[:], inv_c[:], sq[:],
                                start=(ct == 0), stop=(ct == CT - 1),
                            )
                        sdvq = pq3.tile([128, TO], F32, name="sdvq", bufs=1)
                        nc.scalar.activation(sdvq[:], varQ[:], AF.Sqrt, bias=eps_sb[:, 0:1])
                        rstdq = pq.tile([128, TO], F32, name="rstdq")
                        nc.vector.reciprocal(rstdq[:], sdvq[:])
                        cosro = pq.tile([128, TO], F32, name="cosro")
                        nc.vector.tensor_mul(cosro[:], co_sb[:], rstdq[:])
                        sinro = pq.tile([128, TO], F32, name="sinro")
                        nc.vector.tensor_mul(sinro[:], so_sb[:], rstdq[:])

                        # score scaling for all 32 heads on own tokens (rstd
                        # applied to the raw matmul before bias+relu)
                        for m in range(NHF):
                            swt = pq2.tile([128, CT, 128], BF16, name=f"swt_{m}", tag="wQS")
                            nc.scalar.dma_start(
                                swt[:], wQS[NHF + m].rearrange("p (ct j) -> p ct j", j=128))
                            scP = psQ.tile([128, TO], F32, name=f"scP_{m}", tag="scP", bufs=2)
                            for ct in range(CT):
                                nc.tensor.matmul(
                                    scP[:], swt[:, ct, :], xbo[:, ct, :],
                                    start=(ct == 0), stop=(ct == CT - 1),
                                )
                            scs = pq3.tile([128, TO], F32, name=f"scs_{m}", tag="scs", bufs=2)
                            nc.vector.tensor_mul(scs[:], scP[:], rstdq[:])
                            rel = pq3.tile([128, TO], BF16, name=f"rel_{m}", tag="rel", bufs=2)
                            nc.scalar.activation(rel[:], scs[:], AF.Relu, bias=sb_sb[:, m:m + 1])
                            mscP = psQ.tile([128, TO], F32, name=f"mscP_{m}", tag="mscP", bufs=2)
                            nc.tensor.matmul(mscP[:], mean_sf[:], rel[:], start=True, stop=True)
                            nc.scalar.activation(scbc[:, m, :], mscP[:], AF.Copy)

                        # Q for all 32 heads on own tokens, rope/rstd/scale folded
                        for m in range(NHF):
                            qwt = pq2.tile([128, CT, 128], BF16, name=f"qwt_{m}", tag="wQS")
                            nc.scalar.dma_start(
                                qwt[:], wQS[m].rearrange("p (ct j) -> p ct j", j=128))
                            qP = psQ.tile([128, TO], F32, name=f"qP_{m}", tag="qP", bufs=3)
                            for ct in range(CT):
                                nc.tensor.matmul(
                                    qP[:], qwt[:, ct, :], xbo[:, ct, :],
                                    start=(ct == 0), stop=(ct == CT - 1),
                                )
                            raw = pq3.tile([128, TO], F32, name=f"rawq_{m}", tag="rawq", bufs=2)
                            nc.scalar.activation(raw[:], qP[:], AF.Copy)
                            rot = pq3.tile([128, TO], F32, name=f"rotq_{m}", tag="rotq", bufs=2)
                            nc.sync.dma_start(rot[0:64, :], raw[64:128, :])
                            nc.sync.dma_start(rot[64:128, :], raw[0:64, :])
                            t1 = pq3.tile([128, TO], F32, name=f"t1q_{m}", tag="t1q", bufs=2)
                            nc.vector.tensor_mul(t1[:], raw[:], cosro[:])
                            t2 = pq3.tile([128, TO], F32, name=f"t2q_{m}", tag="t2q", bufs=2)
                            nc.vector.tensor_mul(t2[:], rot[:], sinro[:])
                            rs = pq3.tile([128, TO], F32, name=f"rsq_{m}", tag="rsq", bufs=2)
                            nc.vector.tensor_add(rs[:], t1[:], t2[:])
                            nc.vector.tensor_mul(Q_sb[:, m, :], rs[:], scbc[:, m, :])

                    # ---- Phase B: attention, 32 heads x own 256 queries -------
                    with (
                        tc.tile_pool(name="pkv", bufs=1) as pkv,
                        tc.tile_pool(name="pb", bufs=3) as pb,
                        tc.tile_pool(name="psB", bufs=1, space="PSUM") as psB,
                    ):
                        K_sb = pkv.tile([128, NG, T], BF16, name="K_sb")
                        V_sb = pkv.tile([128, NG, KT, 128], BF16, name="V_sb")
                        for g in range(NG):
                            nc.sync.dma_start(K_sb[:, g, :], kv_out[2 * g])
                            nc.sync.dma_start(
                                V_sb[:, g, :, :],
                                kv_out[2 * g + 1].rearrange("p (kt j) -> p kt j", j=128))
                        for h in range(NHF):
                            g = h // 4
                            yP = psB.tile([128, TO], F32, name=f"yP_{h}", tag="yP", bufs=2)
                            csP = psB.tile([128, TO], F32, name=f"csP_{h}", tag="csP", bufs=2)
                            # scores run 2 tiles ahead of the yP/csP consumers so
                            # the PE never waits on the ACT-engine exp latency
                            Es = {}
                            for kt in range(KT + 2):
                                if kt < KT:
                                    sP = psB.tile([128, TO], F32, name=f"sP_{h}_{kt}", tag="sP", bufs=3)
                                    nc.tensor.matmul(
                                        sP[:], K_sb[:, g, kt * 128:(kt + 1) * 128],
                                        Q_sb[:, h, :], start=True, stop=True,
                                    )
                                    E = pb.tile([128, TO], BF16, name=f"E_{h}_{kt}", tag="E", bufs=4)
                                    nc.scalar.activation(E[:], sP[:], AF.Exp)
                                    Es[kt] = E
                                kc = kt - 2
                                if kc >= 0:
                                    E = Es.pop(kc)
                                    nc.tensor.matmul(
                                        yP[:], V_sb[:, g, kc, :], E[:],
                                        start=(kc == 0), stop=(kc == KT - 1),
                                    )
                                    nc.tensor.matmul(
                                        csP[:], ones128[:], E[:],
                                        start=(kc == 0), stop=(kc == KT - 1),
                                    )
                            cs = pb.tile([128, TO], F32, name=f"cs_{h}", tag="cs", bufs=2)
                            nc.scalar.activation(cs[:], csP[:], AF.Copy)
                            rb = pb.tile([128, TO], F32, name=f"rb_{h}", tag="rb", bufs=2)
                            nc.vector.reciprocal(rb[:], cs[:])
                            nc.vector.tensor_mul(yT_sb[:, h, :], yP[:], rb[:])

                    # ---- Phase C+D: proj (feature-major) + x1 + rmsnorm2 ------
                    with (
                        tc.tile_pool(name="pd2", bufs=2) as pd2,
                        tc.tile_pool(name="pdc", bufs=3) as pdc,
                        tc.tile_pool(name="psC", bufs=1, space="PSUM") as psC,
                    ):
                        varP2 = None
                        sq2_t = {}
                        for ct in range(CT):
                            if ct < 4:
                                pw_ap = proj_pre[:, ct]
                            else:
                                pwt = pd2.tile([128, NHF, 128], BF16, name=f"pwt_{ct}", tag="pwt")
                                nc.sync.dma_start(pwt[:], projT[:, :, ct * 128:(ct + 1) * 128])
                                pw_ap = pwt
                            hP = psC.tile([128, TO], F32, name=f"hP_{ct}", tag="hP", bufs=3)
                            for hh in range(NHF):
                                nc.tensor.matmul(
                                    hP[:], pw_ap[:, hh, :], yT_sb[:, hh, :],
                                    start=(hh == 0), stop=(hh == NHF - 1),
                                )
                            nc.vector.tensor_add(x1T[:, ct, :], xow[:, ct, :], hP[:])
                            sq2 = pdc.tile([128, TO], BF16, name=f"sq2_{ct}", tag="sq2")
                            nc.vector.tensor_mul(sq2[:], x1T[:, ct, :], x1T[:, ct, :])
                            sq2_t[ct] = sq2
                            if ct >= 1:
                                if varP2 is None:
                                    varP2 = psC.tile([128, TO], F32, name="varP2", bufs=1)
                                nc.tensor.matmul(
                                    varP2[:], inv_c[:], sq2_t.pop(ct - 1)[:],
                                    start=(ct == 1), stop=False,
                                )
                        nc.tensor.matmul(
                            varP2[:], inv_c[:], sq2_t.pop(CT - 1)[:],
                            start=False, stop=True,
                        )
                        sdv2 = pdc.tile([128, TO], F32, name="sdv2", bufs=1)
                        nc.scalar.activation(sdv2[:], varP2[:], AF.Sqrt, bias=eps_sb[:, 0:1])
                        rstd2 = pdc.tile([128, TO], F32, name="rstd2", bufs=1)
                        nc.vector.reciprocal(rstd2[:], sdv2[:])
                        for ct in range(CT):
                            nc.vector.tensor_mul(n2T[:, ct, :], x1T[:, ct, :], rstd2[:])

            # ---- Phase E: full-ffn SwiGLU MLP for own tokens --------------
            with (
                tc.tile_pool(name="pe", bufs=1) as pe,
                tc.tile_pool(name="pd3", bufs=3) as pd3,
                tc.tile_pool(name="psD", bufs=2, space="PSUM") as psD,
            ):
                    sg = pe.tile([128, FT, TO], BF16, name="sg")
                    for f in range(FT):
                        gw = pe.tile([128, CT, 128], BF16, name=f"gw_{f}", tag="gw", bufs=2)
                        nc.scalar.dma_start(gw[:], gate_w[f].rearrange("p (ct j) -> p ct j", j=128))
                        uw = pe.tile([128, CT, 128], BF16, name=f"uw_{f}", tag="uw", bufs=2)
                        nc.scalar.dma_start(uw[:], up_w[f].rearrange("p (ct j) -> p ct j", j=128))
                        gP = psD.tile([128, TO], F32, name=f"gP_{f}", tag="gP", bufs=2)
                        uP = psD.tile([128, TO], F32, name=f"uP_{f}", tag="uP", bufs=2)
                        for ct in range(CT):
                            nc.tensor.matmul(
                                gP[:], gw[:, ct, :], n2T[:, ct, :],
                                start=(ct == 0), stop=(ct == CT - 1),
                            )
                        for ct in range(CT):
                            nc.tensor.matmul(
                                uP[:], uw[:, ct, :], n2T[:, ct, :],
                                start=(ct == 0), stop=(ct == CT - 1),
                            )
                        sig = pd3.tile([128, TO], BF16, name=f"sig_{f}", tag="sig")
                        nc.scalar.activation(sig[:], gP[:], AF.Sigmoid)
                        m1 = pd3.tile([128, TO], BF16, name=f"m1_{f}", tag="m1")
                        nc.vector.tensor_mul(m1[:], gP[:], sig[:])
                        nc.vector.tensor_mul(sg[:, f, :], m1[:], uP[:])

                    for o in range(CT):
                        dw = pe.tile([128, FT, 128], BF16, name=f"dw_{o}", tag="dw", bufs=2)
                        nc.scalar.dma_start(dw[:], down_w[o].rearrange("p (ft c) -> p ft c", c=128))
                        dP = psD.tile([128, TO], F32, name=f"dP_{o}", tag="dP", bufs=3)
                        for f in range(FT):
                            nc.tensor.matmul(
                                dP[:], dw[:, f, :], sg[:, f, :],
                                start=(f == 0), stop=(f == FT - 1),
                            )
                        ob = pd3.tile([128, TO], F32, name=f"ob_{o}", tag="ob", bufs=2)
                        nc.vector.tensor_add(ob[:], x1T[:, o, :], dP[:])
                        nc.sync.dma_start(outT[o * 128:(o + 1) * 128, :], ob[:])

    nc.compile()
    return nc


def _prep_inputs(inputs):
    x = np.asarray(inputs["x"], np.float32)[0]          # [T, C]
    cos = np.asarray(inputs["cos"], np.float32)
    sin = np.asarray(inputs["sin"], np.float32)
    w1 = np.asarray(inputs["norm1_w"], np.float32)
    w2 = np.asarray(inputs["norm2_w"], np.float32)
    attn_w = np.asarray(inputs["attn_w"], np.float32)
    proj_w = np.asarray(inputs["proj_w"], np.float32)
    scale_w = np.asarray(inputs["scale_w"], np.float32)
    scale_b = np.asarray(inputs["scale_b"], np.float32)
    gate_w = np.asarray(inputs["gate_w"], np.float32)
    up_w = np.asarray(inputs["up_w"], np.float32)
    down_w = np.asarray(inputs["down_w"], np.float32)

    xT = np.ascontiguousarray(x.T)                      # [C, T]
    xT_bf = xT.astype(BF)
    cosT = np.ascontiguousarray(cos.T)                  # [128, T]
    sinTs = sin.T.copy()
    sinTs[0:64] *= -1.0                                 # sign-folded rot half
    sinTs = np.ascontiguousarray(sinTs)

    def lhst_tiles(w, nt):  # [nt*128, C] -> [nt, 128, C] lhsT tile layout
        return np.ascontiguousarray(
            w.reshape(nt, 128, CT, 128).transpose(0, 3, 2, 1).reshape(nt, 128, C)
        ).astype(BF)

    # all q head rows (head h = group h//4, sub q h%4) then all scale rows
    q_rows = np.concatenate(
        [attn_w[(h // 4) * 768 + (h % 4) * 128: (h // 4) * 768 + (h % 4) * 128 + 128]
         for h in range(NHF)], axis=0)
    wqs_dev = lhst_tiles(
        np.concatenate([q_rows, scale_w], axis=0) * w1[None, :], 2 * NHF)

    sb_dev = np.ascontiguousarray(scale_b.reshape(NHF, 128).T)
    # projT[d, h, c] = proj_w[c, h*128+d]
    pw_dev = np.ascontiguousarray(
        proj_w.reshape(C, NHF, 128).transpose(2, 1, 0)
    ).astype(BF)

    g_dev = lhst_tiles(gate_w * w2[None, :], FT)
    u_dev = lhst_tiles(up_w * w2[None, :], FT)
    d_dev = np.ascontiguousarray(
        down_w.reshape(CT, 128, FT, 128).transpose(0, 3, 2, 1).reshape(CT, 128, FT * 128)
    ).astype(BF)

    maps = []
    for g in range(N_CORES):
        osl = slice(g * TO, (g + 1) * TO)
        kv_rows = np.concatenate(
            [attn_w[g * 768 + 512: g * 768 + 640],
             attn_w[g * 768 + 640: g * 768 + 768]], axis=0)
        wkv_dev = lhst_tiles(kv_rows * w1[None, :], 2)
        maps.append({
            "xT": xT_bf,
            "xT_own": np.ascontiguousarray(xT[:, osl]),
            "cosT": cosT,
            "sinT": sinTs,
            "cos_own": np.ascontiguousarray(cosT[:, osl]),
            "sin_own": np.ascontiguousarray(sinTs[:, osl]),
            "wKV": wkv_dev,
            "wQS": wqs_dev,
            "scale_b": sb_dev,
            "projT": pw_dev,
            "gate_w": g_dev,
            "up_w": u_dev,
            "down_w": d_dev,
        })
    return maps


def _run(inputs, **kw):
    if "nc" not in _CACHE:
        _CACHE["nc"] = _build()
    nc = _CACHE["nc"]
    maps = _prep_inputs(inputs)
    res = run_bass_kernel_spmd(nc, maps, core_ids=list(range(N_CORES)), **kw)
    full = np.concatenate([res.results[g]["outT"] for g in range(N_CORES)], axis=1)
    out = np.ascontiguousarray(full.T)[None].astype(np.float32)
    return out, res


def kernel(**inputs):
    out, _ = _run(inputs)
    return out


def kernel_traced(**inputs):
    out, res = _run(inputs, trace=True)
    return out, res



# revision 32
# speedup vs baseline: 3.4408x; 3.4408x over previous
"""Dense transformer block on 8 trn2 cores: fp8 attention path + ffn-sharded MLP.

Structure (per core, token slice TO=256 own tokens, ffn slice FT_LOC=11 tiles):
  A1  full-T rmsnorm1 stats + K/V for own GQA group (fp8 DoubleRow matmuls),
      published with one small AllGather (bf16) overlapped with A2.
  A2  own-token rmsnorm1 -> score-scaling + Q for all 32 heads (fp8 DoubleRow).
  B   attention, 32 heads x own 256 queries: scores bf16, exp -> fp8 E tiles
      (bias -6ln2 rescale), AV + denominator as fp8 DoubleRow over key pairs.
  C   proj (fp8 DoubleRow over head pairs) + residual + rmsnorm2 for own
      tokens; n2 published with two chunked AllGathers (bf16).
  E   ffn-sharded SwiGLU: this core's 11 ffn tiles over ALL 2048 tokens in
      two 1024-token halves; partial down-proj outputs ReduceScattered (add,
      bf16) back to token owners; final residual add + store.

fp8 scaling scheme (e4m3, all power-of-2 so exactly invertible):
  x is fed as fp8(x*16); qkv/scale/proj weights as fp8(w*64); the 2^-10
  product fold rides the rope cos/sin tables (K,Q), an rstd*2^-10 tile (V,
  scaling), or the fused (hP*2^-10 + x) residual add (proj).  E = exp(s-6ln2)
  keeps the fp8 range safe (scores empirically within +-4); the softmax
  denominator uses a 1/16 stationary so yP/cs lands at 16*y, which is the
  fp8-friendly scale proj consumes.  MLP stays bf16: fp8 there fails the
  2e-2 gate (measured 3.4e-2 for any single fp8 MLP matmul).

Why: the baseline streamed 270 MB of MLP weights per core (token-parallel
full-ffn MLP) which saturated DMA (70% busy) alongside PE (83%), and issued
27k PE instructions whose 71ns sequencer cost throttled the free=256
matmuls.  This version cuts per-core weight traffic to ~110 MB, halves the
attention-path PE cycles (DoubleRow), halves instruction count, and runs
MLP matmuls at free=1024.
"""

import numpy as np
import ml_dtypes

import concourse.mybir as mybir
import concourse.tile as tile
from concourse import bacc
from concourse.bass_utils import run_bass_kernel_spmd

BF = ml_dtypes.bfloat16
F8 = ml_dtypes.float8_e4m3
F32 = mybir.dt.float32
BF16 = mybir.dt.bfloat16
FP8 = mybir.dt.float8e4
AF = mybir.ActivationFunctionType
DR = mybir.MatmulPerfMode.DoubleRow
ALU = mybir.AluOpType

N_CORES = 8
T = 2048
C = 4096
HS = 128
NHF = 32          # full q heads
NG = 8            # kv groups
CT = C // 128     # 32 c-tiles
TCH = 512         # token chunk, phase A full-T pass
NCH = T // TCH
KT = T // 128     # 16 key tiles
FT = 86           # full ffn tiles (86*128 = 11008)
FT_LOC = 11       # per-core ffn tiles (6 cores x11 + 2 cores x10, zero-pad)
TO = T // N_CORES  # 256 own tokens
TH = 1024          # MLP token half
EPS = 1e-5
SF = 1.0 / float(np.sqrt(HS))
NEG6LN2 = float(-6.0 * np.log(2.0))
SC10 = float(2.0 ** -10)   # undo x*16 * w*64

_CACHE = {}


def _build(cc=True):
    nc = bacc.Bacc(None, target_bir_lowering=False, num_devices=N_CORES)

    xT8 = nc.dram_tensor("xT8", [C, T], FP8, kind="ExternalInput")
    x8_own = nc.dram_tensor("x8_own", [C, TO], FP8, kind="ExternalInput")
    xT_own = nc.dram_tensor("xT_own", [C, TO], F32, kind="ExternalInput")
    cosT = nc.dram_tensor("cosT", [128, T], F32, kind="ExternalInput")
    sinT = nc.dram_tensor("sinT", [128, T], F32, kind="ExternalInput")
    cos_own = nc.dram_tensor("cos_own", [128, TO], F32, kind="ExternalInput")
    sin_own = nc.dram_tensor("sin_own", [128, TO], F32, kind="ExternalInput")
    # K, V, rotated-K tiles for this core's own group, w1-folded, *64 fp8
    wKV = nc.dram_tensor("wKV", [3, 128, C], FP8, kind="ExternalInput")
    # all 32 q head tiles then all 32 scale tiles, w1-folded, *64 fp8
    wQS = nc.dram_tensor("wQS", [2 * NHF, 128, C], FP8, kind="ExternalInput")
    scale_b = nc.dram_tensor("scale_b", [128, NHF], F32, kind="ExternalInput")
    projT = nc.dram_tensor("projT", [CT, 128, NHF * 128], FP8, kind="ExternalInput")
    gate_w = nc.dram_tensor("gate_w", [FT_LOC, 128, C], BF16, kind="ExternalInput")
    up_w = nc.dram_tensor("up_w", [FT_LOC, 128, C], BF16, kind="ExternalInput")
    down_w = nc.dram_tensor("down_w", [CT, 128, FT_LOC * 128], BF16,
                            kind="ExternalInput")
    outT = nc.dram_tensor("outT", [C, TO], F32, kind="ExternalOutput")

    with tile.TileContext(nc) as tc:
        with (
            tc.tile_pool(name="persist", bufs=1) as pp,
            tc.tile_pool(name="pdx", bufs=1) as pdx,
            tc.tile_pool(name="dram", bufs=1, space="DRAM") as dram,
        ):
            # 1/(C*256): A1 var uses fp8 x*16 squares (256*x^2)
            inv_c1 = pp.tile([128, 128], BF16, name="inv_c1")
            nc.vector.memset(inv_c1[:], 2.0 ** -20)
            # 1/C: A2/C var from f32 x
            inv_c2 = pp.tile([128, 128], BF16, name="inv_c2")
            nc.vector.memset(inv_c2[:], 1.0 / C)
            mean_sf = pp.tile([128, 128], BF16, name="mean_sf")
            nc.vector.memset(mean_sf[:], SF / HS)
            ones8 = pp.tile([128, 2, 128], FP8, name="ones8")
            nc.vector.memset(ones8[:], 1.0 / 16.0)
            eps_sb = pp.tile([128, 1], F32, name="eps_sb")
            nc.vector.memset(eps_sb[:], EPS)
            nl2_sb = pp.tile([128, 1], F32, name="nl2_sb")
            nc.vector.memset(nl2_sb[:], NEG6LN2)

            kv_in = dram.tile([2, 128, T], BF16, name="kv_in")
            kv_out = dram.tile([2 * NG, 128, T], BF16, name="kv_out",
                               addr_space="Shared")
            n2_in = [dram.tile([128, CT * 128], BF16, name=f"n2_in{h}")
                     for h in range(2)]
            n2_ag = [dram.tile([N_CORES, 128, CT * 128], BF16, name=f"n2_ag{h}",
                               addr_space="Shared") for h in range(2)]
            part = [dram.tile([N_CORES, 128, CT * 128], BF16, name=f"part{h}")
                    for h in range(2)]
            rs_out = [dram.tile([128, CT * 128], BF16, name=f"rs_out{h}")
                      for h in range(2)]

            x1T = pdx.tile([128, CT, TO], F32, name="x1T")

            with tc.tile_pool(name="pqkv", bufs=1) as pqkv:
                # ---- Phase A1: full-T rmsnorm1 -> K (rope) and V ----------
                with (
                    tc.tile_pool(name="pa", bufs=1) as pa,
                    tc.tile_pool(name="pa3", bufs=3) as pa3,
                    tc.tile_pool(name="psA", bufs=1, space="PSUM") as psA,
                ):
                    # K/V/rotK weights are chunk-invariant: load once (flat
                    # 2-dim DMA views keep descriptor runs at 4KB)
                    kwt = pa.tile([128, CT, 128], FP8, name="kwt")
                    nc.scalar.dma_start(
                        kwt[:].rearrange("p ct j -> p (ct j)"), wKV[0])
                    vwt = pa.tile([128, CT, 128], FP8, name="vwt")
                    nc.scalar.dma_start(
                        vwt[:].rearrange("p ct j -> p (ct j)"), wKV[1])
                    rwt = pa.tile([128, CT, 128], FP8, name="rwt")
                    nc.scalar.dma_start(
                        rwt[:].rearrange("p ct j -> p (ct j)"), wKV[2])
                    # rmsnorm1 scaling commutes with the C-contraction, so
                    # K/V matmul RAW fp8 x*16 and rstd (and the 2^-10 fp8
                    # fold) ride the rope cos/sin (K) or one output mul (V).
                    for ci in range(NCH):
                        t0 = ci * TCH
                        tsl = slice(t0, t0 + TCH)
                        xbf = pa.tile([128, CT, TCH], FP8, name=f"xbf_{ci}", tag="xbf", bufs=2)
                        cos_sb = pa3.tile([128, TCH], F32, name=f"cos_{ci}", tag="cos", bufs=2)
                        nc.scalar.dma_start(cos_sb[:], cosT[:, tsl])
                        sin_sb = pa3.tile([128, TCH], F32, name=f"sin_{ci}", tag="sin", bufs=2)
                        nc.scalar.dma_start(sin_sb[:], sinT[:, tsl])
                        varP = psA.tile([128, TCH], F32, name=f"varP_{ci}", tag="varP", bufs=2)
                        for cq in range(4):
                            nc.sync.dma_start(
                                xbf[:, cq * 8:(cq + 1) * 8, :],
                                xT8[cq * 1024:(cq + 1) * 1024, tsl].rearrange(
                                    "(ct p) t -> p ct t", p=128))
                        for ct in range(CT):
                            sq = pa3.tile([128, TCH], BF16, name=f"sq_{ci}_{ct}", tag="sq", bufs=3)
                            nc.vector.tensor_mul(sq[:], xbf[:, ct, :], xbf[:, ct, :])
                            nc.tensor.matmul(
                                varP[:], inv_c1[:], sq[:],
                                start=(ct == 0), stop=(ct == CT - 1),
                            )
                        sdv = pa3.tile([128, TCH], F32, name=f"sdv_{ci}", tag="sdv", bufs=2)
                        nc.scalar.activation(sdv[:], varP[:], AF.Sqrt, bias=eps_sb[:, 0:1])
                        rstd = pa3.tile([128, TCH], F32, name=f"rstd_{ci}", tag="rstd", bufs=2)
                        nc.vector.reciprocal(rstd[:], sdv[:])

                        # cos/sin tables carry the 2^-10 fp8 fold from host
                        cosr = pa3.tile([128, TCH], F32, name=f"cosr_{ci}", tag="cosr", bufs=2)
                        nc.vector.tensor_mul(cosr[:], cos_sb[:], rstd[:])
                        sinr = pa3.tile([128, TCH], F32, name=f"sinr_{ci}", tag="sinr", bufs=2)
                        nc.vector.tensor_mul(sinr[:], sin_sb[:], rstd[:])
                        rstd10 = pa3.tile([128, TCH], F32, name=f"rstd10_{ci}", tag="rstd10", bufs=2)
                        nc.vector.tensor_scalar_mul(rstd10[:], rstd[:], SC10)

                        # K and rotated-K for own group, fp8 DoubleRow: rope
                        # rotation baked into a second stationary set, so no
                        # SBUF-SBUF partition-shift DMAs on the critical path
                        kP = psA.tile([128, TCH], F32, name=f"kP_{ci}", tag="kP", bufs=2)
                        for cp in range(CT // 2):
                            nc.tensor.matmul(
                                kP[:], kwt[:, 2 * cp:2 * cp + 2, :],
                                xbf[:, 2 * cp:2 * cp + 2, :],
                                start=(cp == 0), stop=(cp == CT // 2 - 1),
                                perf_mode=DR,
                            )
                        rP = psA.tile([128, TCH], F32, name=f"rP_{ci}", tag="rP", bufs=2)
                        for cp in range(CT // 2):
                            nc.tensor.matmul(
                                rP[:], rwt[:, 2 * cp:2 * cp + 2, :],
                                xbf[:, 2 * cp:2 * cp + 2, :],
                                start=(cp == 0), stop=(cp == CT // 2 - 1),
                                perf_mode=DR,
                            )
                        vP = psA.tile([128, TCH], F32, name=f"vP_{ci}", tag="vP", bufs=2)
                        for cp in range(CT // 2):
                            nc.tensor.matmul(
                                vP[:], vwt[:, 2 * cp:2 * cp + 2, :],
                                xbf[:, 2 * cp:2 * cp + 2, :],
                                start=(cp == 0), stop=(cp == CT // 2 - 1),
                                perf_mode=DR,
                            )
                        t1 = pa3.tile([128, TCH], F32, name=f"t1_{ci}", tag="t1", bufs=2)
                        nc.vector.tensor_mul(t1[:], kP[:], cosr[:])
                        t2 = pa3.tile([128, TCH], F32, name=f"t2_{ci}", tag="t2", bufs=2)
                        nc.vector.tensor_mul(t2[:], rP[:], sinr[:])
                        ko = pa3.tile([128, TCH], BF16, name=f"ko_{ci}", tag="ko", bufs=2)
                        nc.vector.tensor_add(ko[:], t1[:], t2[:])
                        nc.scalar.dma_start(kv_in[0, :, tsl], ko[:])

                        vsc = pa3.tile([128, TCH], BF16, name=f"vsc_{ci}", tag="vsc", bufs=2)
                        nc.vector.tensor_mul(vsc[:], vP[:], rstd10[:])
                        for tt in range(TCH // 128):
                            vtk = pa3.tile([128, 128], BF16, name=f"vtk_{ci}_{tt}", tag="vtk", bufs=2)
                            nc.scalar.dma_start(
                                vtk[:], vsc[:, tt * 128:(tt + 1) * 128], transpose=True)
                            nc.scalar.dma_start(
                                kv_in[1, :, (ci * (TCH // 128) + tt) * 128:
                                      (ci * (TCH // 128) + tt + 1) * 128],
                                vtk[:])

                    if cc:
                        nc.gpsimd.collective_compute(
                            "AllGather", ALU.bypass,
                            replica_groups=[list(range(N_CORES))],
                            ins=[kv_in.opt()], outs=[kv_out.opt()],
                        )

                with tc.tile_pool(name="ppre", bufs=1) as ppre:
                    Q_sb = ppre.tile([128, NHF, TO], BF16, name="Q_sb")
                    yT8 = ppre.tile([128, NHF, TO], FP8, name="yT8")
                    proj_pre = ppre.tile([128, 4, NHF, 128], FP8, name="proj_pre")
                    for pc in range(4):
                        nc.sync.dma_start(
                            proj_pre[:, pc].rearrange("p h j -> p (h j)"), projT[pc])
                    # ---- Phase A2+B fused: per head, scaling -> Q -> attention.
                    # All pools coexist so the engine queues interleave the two
                    # pipelines; K/V load up front right after the AllGather.
                    with (
                        tc.tile_pool(name="pkv", bufs=1) as pkv,
                        tc.tile_pool(name="pq", bufs=1) as pq,
                        tc.tile_pool(name="pq2", bufs=2) as pq2,
                        tc.tile_pool(name="pq3", bufs=3) as pq3,
                        tc.tile_pool(name="pb", bufs=3) as pb,
                        tc.tile_pool(name="psQ", bufs=1, space="PSUM") as psQ,
                    ):
                        K_sb = pkv.tile([128, NG, T], BF16, name="K_sb")
                        V8 = pkv.tile([128, NG, KT, 128], FP8, name="V8")
                        with tc.tile_pool(name="pvst", bufs=2) as pvst:
                            for g in range(NG):
                                nc.sync.dma_start(K_sb[:, g, :], kv_out[2 * g])
                                vst = pvst.tile([128, KT, 128], BF16, name=f"vst_{g}", tag="vst")
                                nc.sync.dma_start(
                                    vst[:].rearrange("p kt j -> p (kt j)"), kv_out[2 * g + 1])
                                nc.vector.tensor_copy(V8[:, g], vst[:])

                        scbc = pq.tile([128, NHF, TO], BF16, name="scbc")
                        sb_sb = pq.tile([128, NHF], F32, name="sb_sb")
                        nc.sync.dma_start(sb_sb[:], scale_b[:])
                        co_sb = pq.tile([128, TO], F32, name="co_sb")
                        nc.sync.dma_start(co_sb[:], cos_own[:])
                        so_sb = pq.tile([128, TO], F32, name="so_sb")
                        nc.sync.dma_start(so_sb[:], sin_own[:])
                        xbo = pq.tile([128, CT, TO], FP8, name="xbo")
                        nc.sync.dma_start(
                            xbo[:], x8_own.rearrange("(ct p) t -> p ct t", p=128))

                        # rmsnorm1 stats for own tokens from the fp8 x*16
                        # squares (256 x^2 -> inv_c1 stationary), so the f32
                        # x slice is only needed later in phase C
                        varQ = psQ.tile([128, TO], F32, name="varQ")
                        for ct in range(CT):
                            sq = pq3.tile([128, TO], BF16, name=f"sqq_{ct}", tag="sqq")
                            nc.vector.tensor_mul(sq[:], xbo[:, ct, :], xbo[:, ct, :])
                            nc.tensor.matmul(
                                varQ[:], inv_c1[:], sq[:],
                                start=(ct == 0), stop=(ct == CT - 1),
                            )
                        sdvq = pq3.tile([128, TO], F32, name="sdvq", bufs=1)
                        nc.scalar.activation(sdvq[:], varQ[:], AF.Sqrt, bias=eps_sb[:, 0:1])
                        rstdq = pq.tile([128, TO], F32, name="rstdq")
                        nc.vector.reciprocal(rstdq[:], sdvq[:])
                        rstdq10 = pq.tile([128, TO], F32, name="rstdq10")
                        nc.vector.tensor_scalar_mul(rstdq10[:], rstdq[:], SC10)
                        cosro = pq.tile([128, TO], F32, name="cosro")
                        nc.vector.tensor_mul(cosro[:], co_sb[:], rstdq[:])
                        sinro = pq.tile([128, TO], F32, name="sinro")
                        nc.vector.tensor_mul(sinro[:], so_sb[:], rstdq[:])

                        for m in range(NHF):
                            g = m // 4
                            # score scaling for head m
                            swt = pq2.tile([128, CT, 128], FP8, name=f"swt_{m}", tag="wQS", bufs=6)
                            nc.scalar.dma_start(
                                swt[:].rearrange("p ct j -> p (ct j)"), wQS[NHF + m])
                            scP = psQ.tile([128, TO], F32, name=f"scP_{m}", tag="scP", bufs=1)
                            for cp in range(CT // 2):
                                nc.tensor.matmul(
                                    scP[:], swt[:, 2 * cp:2 * cp + 2, :],
                                    xbo[:, 2 * cp:2 * cp + 2, :],
                                    start=(cp == 0), stop=(cp == CT // 2 - 1),
                                    perf_mode=DR,
                                )
                            scs = pq3.tile([128, TO], F32, name=f"scs_{m}", tag="scs", bufs=2)
                            nc.vector.tensor_mul(scs[:], scP[:], rstdq10[:])
                            rel = pq3.tile([128, TO], BF16, name=f"rel_{m}", tag="rel", bufs=2)
                            nc.scalar.activation(rel[:], scs[:], AF.Relu, bias=sb_sb[:, m:m + 1])
                            mscP = psQ.tile([128, TO], F32, name=f"mscP_{m}", tag="mscP", bufs=2)
                            nc.tensor.matmul(mscP[:], mean_sf[:], rel[:], start=True, stop=True)
                            nc.scalar.activation(scbc[:, m, :], mscP[:], AF.Copy)

                            # Q for head m, rope/rstd/scale folded
                            qwt = pq2.tile([128, CT, 128], FP8, name=f"qwt_{m}", tag="wQS", bufs=6)
                            nc.sync.dma_start(
                                qwt[:].rearrange("p ct j -> p (ct j)"), wQS[m])
                            qP = psQ.tile([128, TO], F32, name=f"qP_{m}", tag="qP", bufs=1)
                            for cp in range(CT // 2):
                                nc.tensor.matmul(
                                    qP[:], qwt[:, 2 * cp:2 * cp + 2, :],
                                    xbo[:, 2 * cp:2 * cp + 2, :],
                                    start=(cp == 0), stop=(cp == CT // 2 - 1),
                                    perf_mode=DR,
                                )
                            raw = pq3.tile([128, TO], F32, name=f"rawq_{m}", tag="rawq", bufs=2)
                            nc.scalar.activation(raw[:], qP[:], AF.Copy)
                            rot = pq3.tile([128, TO], F32, name=f"rotq_{m}", tag="rotq", bufs=2)
                            nc.sync.dma_start(rot[0:64, :], raw[64:128, :])
                            nc.sync.dma_start(rot[64:128, :], raw[0:64, :])
                            t1 = pq3.tile([128, TO], F32, name=f"t1q_{m}", tag="t1q", bufs=2)
                            nc.vector.tensor_mul(t1[:], raw[:], cosro[:])
                            t2 = pq3.tile([128, TO], F32, name=f"t2q_{m}", tag="t2q", bufs=2)
                            nc.vector.tensor_mul(t2[:], rot[:], sinro[:])
                            rs = pq3.tile([128, TO], F32, name=f"rsq_{m}", tag="rsq", bufs=2)
                            nc.vector.tensor_add(rs[:], t1[:], t2[:])
                            nc.vector.tensor_mul(Q_sb[:, m, :], rs[:], scbc[:, m, :])

                            # attention for head m over all 2048 keys
                            yP = psQ.tile([128, TO], F32, name=f"yP_{m}", tag="yP", bufs=2)
                            csP = psQ.tile([128, TO], F32, name=f"csP_{m}", tag="csP", bufs=2)
                            Es = {}
                            for p in range(KT // 2 + 1):
                                if p < KT // 2:
                                    E8 = pb.tile([128, 2, TO], FP8, name=f"E_{m}_{p}", tag="E", bufs=3)
                                    sP = psQ.tile([128, 2, TO], F32, name=f"sP_{m}_{p}", tag="sP", bufs=3)
                                    for j in range(2):
                                        kt = 2 * p + j
                                        nc.tensor.matmul(
                                            sP[:, j, :], K_sb[:, g, kt * 128:(kt + 1) * 128],
                                            Q_sb[:, m, :], start=True, stop=True,
                                        )
                                    nc.scalar.activation(E8[:], sP[:], AF.Exp,
                                                         bias=nl2_sb[:, 0:1])
                                    Es[p] = E8
                                pc = p - 1
                                if pc >= 0:
                                    E8 = Es.pop(pc)
                                    nc.tensor.matmul(
                                        yP[:], V8[:, g, 2 * pc:2 * pc + 2, :], E8[:],
                                        start=(pc == 0), stop=(pc == KT // 2 - 1),
                                        perf_mode=DR,
                                    )
                                    nc.tensor.matmul(
                                        csP[:], ones8[:], E8[:],
                                        start=(pc == 0), stop=(pc == KT // 2 - 1),
                                        perf_mode=DR,
                                    )
                            rb = pb.tile([128, TO], F32, name=f"rb_{m}", tag="rb", bufs=2)
                            nc.vector.reciprocal(rb[:], csP[:])
                            nc.vector.tensor_mul(yT8[:, m, :], yP[:], rb[:])

                    # ---- Phase C: proj (fp8 DR) + x1 + rmsnorm2 + n2 publish --
                    with (
                        tc.tile_pool(name="pd2", bufs=2) as pd2,
                        tc.tile_pool(name="pdc", bufs=3) as pdc,
                        tc.tile_pool(name="psC", bufs=1, space="PSUM") as psC,
                    ):
                        n2T = pdc.tile([128, CT, TO], BF16, name="n2T", bufs=1)
                        varP2 = None
                        sq2_t = {}
                        for ct in range(CT):
                            if ct < 4:
                                pw_ap = proj_pre[:, ct]
                            else:
                                pwt = pd2.tile([128, NHF, 128], FP8, name=f"pwt_{ct}", tag="pwt", bufs=4)
                                nc.sync.dma_start(
                                    pwt[:].rearrange("p h j -> p (h j)"), projT[ct])
                                pw_ap = pwt
                            hP = psC.tile([128, TO], F32, name=f"hP_{ct}", tag="hP", bufs=3)
                            for hp in range(NHF // 2):
                                nc.tensor.matmul(
                                    hP[:], pw_ap[:, 2 * hp:2 * hp + 2, :],
                                    yT8[:, 2 * hp:2 * hp + 2, :],
                                    start=(hp == 0), stop=(hp == NHF // 2 - 1),
                                    perf_mode=DR,
                                )
                            # x1 = x + hP * 2^-10 (undo fp8 scales), fused
                            nc.vector.scalar_tensor_tensor(
                                x1T[:, ct, :], hP[:], SC10, xow[:, ct, :],
                                ALU.mult, ALU.add)
                            sq2 = pdc.tile([128, TO], BF16, name=f"sq2_{ct}", tag="sq2")
                            nc.vector.tensor_mul(sq2[:], x1T[:, ct, :], x1T[:, ct, :])
                            sq2_t[ct] = sq2
                            if ct >= 1:
                                if varP2 is None:
                                    varP2 = psC.tile([128, TO], F32, name="varP2", bufs=1)
                                nc.tensor.matmul(
                                    varP2[:], inv_c2[:], sq2_t.pop(ct - 1)[:],
                                    start=(ct == 1), stop=False,
                                )
                        nc.tensor.matmul(
                            varP2[:], inv_c2[:], sq2_t.pop(CT - 1)[:],
                            start=False, stop=True,
                        )
                        sdv2 = pdc.tile([128, TO], F32, name="sdv2", bufs=1)
                        nc.scalar.activation(sdv2[:], varP2[:], AF.Sqrt, bias=eps_sb[:, 0:1])
                        rstd2 = pdc.tile([128, TO], F32, name="rstd2", bufs=1)
                        nc.vector.reciprocal(rstd2[:], sdv2[:])
                        for ct in range(CT):
                            nc.vector.tensor_mul(n2T[:, ct, :], x1T[:, ct, :], rstd2[:])
                        for h in range(2):
                            hsl = slice(h * 128, (h + 1) * 128)
                            nc.scalar.dma_start(
                                n2_in[h].rearrange("p (ct t) -> p ct t", t=128),
                                n2T[:, :, hsl])
                            if cc:
                                nc.gpsimd.collective_compute(
                                    "AllGather", ALU.bypass,
                                    replica_groups=[list(range(N_CORES))],
                                    ins=[n2_in[h].opt()], outs=[n2_ag[h].opt()],
                                )

            # ---- Phase E: ffn-sharded SwiGLU over all T, two halves -------
            with (
                tc.tile_pool(name="pe", bufs=1) as pe,
                tc.tile_pool(name="pd3", bufs=3) as pd3,
                tc.tile_pool(name="psD", bufs=1, space="PSUM") as psD,
            ):
                for h in range(2):
                    # g-major token layout: block g = core g's 128 tokens of
                    # this half; every load lands as one 8KB/partition run
                    n2h = pe.tile([128, N_CORES, CT, 128], BF16, name=f"n2h_{h}", tag="n2h", bufs=1)
                    for g in range(N_CORES):
                        nc.sync.dma_start(
                            n2h[:, g].rearrange("p ct t -> p (ct t)"), n2_ag[h][g])
                    sg = pe.tile([128, FT_LOC, TH], BF16, name=f"sg_{h}", tag="sg", bufs=1)
                    for f in range(FT_LOC):
                        gw = pe.tile([128, CT, 128], BF16, name=f"gw_{h}_{f}", tag="gw", bufs=2)
                        nc.scalar.dma_start(
                            gw[:].rearrange("p ct j -> p (ct j)"), gate_w[f])
                        uw = pe.tile([128, CT, 128], BF16, name=f"uw_{h}_{f}", tag="uw", bufs=2)
                        nc.scalar.dma_start(
                            uw[:].rearrange("p ct j -> p (ct j)"), up_w[f])
                        # PSUM bank = 512 f32: run matmuls in 512-token quarters
                        for q in range(2):
                            qsl = slice(q * 512, (q + 1) * 512)
                            gP = psD.tile([128, 512], F32, name=f"gP_{h}_{f}_{q}", tag="gP", bufs=2)
                            uP = psD.tile([128, 512], F32, name=f"uP_{h}_{f}_{q}", tag="uP", bufs=2)
                            for ct in range(CT):
                                nc.tensor.matmul(
                                    gP[:], gw[:, ct, :], n2h[:, 4 * q:4 * q + 4, ct, :],
                                    start=(ct == 0), stop=(ct == CT - 1),
                                )
                            for ct in range(CT):
                                nc.tensor.matmul(
                                    uP[:], uw[:, ct, :], n2h[:, 4 * q:4 * q + 4, ct, :],
                                    start=(ct == 0), stop=(ct == CT - 1),
                                )
                            sig = pd3.tile([128, 512], BF16, name=f"sig_{h}_{f}_{q}", tag="sig")
                            nc.scalar.activation(sig[:], gP[:], AF.Sigmoid)
                            m1 = pd3.tile([128, 512], BF16, name=f"m1_{h}_{f}_{q}", tag="m1")
                            nc.vector.tensor_mul(m1[:], gP[:], sig[:])
                            nc.vector.tensor_mul(sg[:, f, qsl], m1[:], uP[:])

                    for o in range(CT):
                        dw = pe.tile([128, FT_LOC, 128], BF16, name=f"dw_{h}_{o}", tag="dw", bufs=2)
                        nc.scalar.dma_start(
                            dw[:].rearrange("p ft c -> p (ft c)"), down_w[o])
                        for q in range(2):
                            qsl = slice(q * 512, (q + 1) * 512)
                            dP = psD.tile([128, 512], F32, name=f"dP_{h}_{o}_{q}", tag="dP", bufs=2)
                            for f in range(FT_LOC):
                                nc.tensor.matmul(
                                    dP[:], dw[:, f, :], sg[:, f, qsl],
                                    start=(f == 0), stop=(f == FT_LOC - 1),
                                )
                            pd_ = pd3.tile([128, 512], BF16, name=f"pd_{h}_{o}_{q}", tag="pd", bufs=2)
                            nc.scalar.activation(pd_[:], dP[:], AF.Copy)
                            nc.sync.dma_start(
                                part[h].rearrange("b p (ct t) -> p ct b t", t=128)[
                                    :, o, 4 * q:4 * q + 4, :],
                                pd_.rearrange("p (b t) -> p b t", t=128))
                    if cc:
                        nc.gpsimd.collective_compute(
                            "ReduceScatter", ALU.add,
                            replica_groups=[list(range(N_CORES))],
                            ins=[part[h].opt()], outs=[rs_out[h].opt()],
                        )

                # final: own-token residual add + store, per half
                for h in range(2):
                    hsl = slice(h * 128, (h + 1) * 128)
                    rsb = pd3.tile([128, CT, 128], BF16, name=f"rsb_{h}", tag="rsb", bufs=1)
                    nc.sync.dma_start(
                        rsb[:].rearrange("p ct t -> p (ct t)"), rs_out[h])
                    ob = pd3.tile([128, CT, 128], F32, name=f"ob_{h}", tag="ob", bufs=1)
                    nc.vector.tensor_add(ob[:], x1T[:, :, hsl], rsb[:])
                    nc.sync.dma_start(
                        outT.rearrange("(ct p) t -> p ct t", p=128)[:, :, hsl], ob[:])

    nc.compile()
    return nc


def _prep_inputs(inputs):
    x = np.asarray(inputs["x"], np.float32)[0]          # [T, C]
    cos = np.asarray(inputs["cos"], np.float32)
    sin = np.asarray(inputs["sin"], np.float32)
    w1 = np.asarray(inputs["norm1_w"], np.float32)
    w2 = np.asarray(inputs["norm2_w"], np.float32)
    attn_w = np.asarray(inputs["attn_w"], np.float32)
    proj_w = np.asarray(inputs["proj_w"], np.float32)
    scale_w = np.asarray(inputs["scale_w"], np.float32)
    scale_b = np.asarray(inputs["scale_b"], np.float32)
    gate_w = np.asarray(inputs["gate_w"], np.float32)
    up_w = np.asarray(inputs["up_w"], np.float32)
    down_w = np.asarray(inputs["down_w"], np.float32)

    xT = np.ascontiguousarray(x.T)                      # [C, T]
    xT8 = (np.asarray(xT, BF).astype(np.float32) * 16.0).astype(F8)
    # rope tables carry the 2^-10 fp8 product fold (x*16 . w*64)
    cosT = np.ascontiguousarray(cos.T) * np.float32(SC10)
    # K path: rotation sign baked into the rotated weight rows -> plain sin
    sinT_dev = np.ascontiguousarray(sin.T) * np.float32(SC10)
    # Q path: DMA-shifted halves, sign folded into the table's first half
    sinQ = sin.T.copy()
    sinQ[0:64] *= -1.0
    sinQ = np.ascontiguousarray(sinQ) * np.float32(SC10)

    def lhst_tiles(w, nt, dt):  # [nt*128, C] -> [nt, 128, C] lhsT tile layout
        return np.ascontiguousarray(
            w.reshape(nt, 128, CT, 128).transpose(0, 3, 2, 1).reshape(nt, 128, C)
        ).astype(dt)

    # all q head rows (head h = group h//4, sub q h%4) then all scale rows
    q_rows = np.concatenate(
        [attn_w[(h // 4) * 768 + (h % 4) * 128: (h // 4) * 768 + (h % 4) * 128 + 128]
         for h in range(NHF)], axis=0)
    wqs_dev = lhst_tiles(
        np.concatenate([q_rows, scale_w], axis=0) * w1[None, :] * 64.0, 2 * NHF, F8)

    sb_dev = np.ascontiguousarray(scale_b.reshape(NHF, 128).T)
    # projT[ct, d, h*128+j] = proj_w[ct*128+j, h*128+d], *64 fp8 (ct-major so
    # each per-ct stationary tile load is one contiguous 4KB/partition run)
    pw_dev = np.ascontiguousarray(
        (proj_w * 64.0).reshape(CT, 128, NHF, 128).transpose(0, 3, 2, 1)
        .reshape(CT, 128, NHF * 128)
    ).astype(F8)

    # ffn split: cores 0-5 get 11 tiles, cores 6-7 get 10 (+1 zero pad)
    fsl_start = [0, 11, 22, 33, 44, 55, 66, 76]
    fsl_len = [11, 11, 11, 11, 11, 11, 10, 10]

    g_full = lhst_tiles(gate_w * w2[None, :], FT, BF)
    u_full = lhst_tiles(up_w * w2[None, :], FT, BF)
    # d_full[f, o, j, d] view: standard lhsT per (o, f) block
    d_full = np.ascontiguousarray(
        down_w.reshape(CT, 128, FT, 128).transpose(0, 3, 2, 1)
    ).astype(BF)  # [CT, 128, FT, 128]

    maps = []
    for g in range(N_CORES):
        osl = slice(g * TO, (g + 1) * TO)
        k_rows = attn_w[g * 768 + 512: g * 768 + 640]
        v_rows = attn_w[g * 768 + 640: g * 768 + 768]
        # rotated K rows: rot(K)[d] = -K[d+64] (d<64), K[d-64] (d>=64)
        kr_rows = np.concatenate([-k_rows[64:], k_rows[:64]], axis=0)
        wkv_dev = lhst_tiles(
            np.concatenate([k_rows, v_rows, kr_rows], axis=0) * w1[None, :] * 64.0,
            3, F8)

        s0, ln = fsl_start[g], fsl_len[g]
        g_dev = np.zeros((FT_LOC, 128, C), BF)
        g_dev[:ln] = g_full[s0:s0 + ln]
        u_dev = np.zeros((FT_LOC, 128, C), BF)
        u_dev[:ln] = u_full[s0:s0 + ln]
        d_dev = np.zeros((CT, 128, FT_LOC, 128), BF)
        d_dev[:, :, :ln] = d_full[:, :, s0:s0 + ln]
        d_dev = np.ascontiguousarray(d_dev.reshape(CT, 128, FT_LOC * 128))

        maps.append({
            "xT8": xT8,
            "x8_own": np.ascontiguousarray(xT8[:, osl]),
            "xT_own": np.ascontiguousarray(xT[:, osl]),
            "cosT": cosT,
            "sinT": sinT_dev,
            "cos_own": np.ascontiguousarray(cosT[:, osl]),
            "sin_own": np.ascontiguousarray(sinQ[:, osl]),
            "wKV": wkv_dev,
            "wQS": wqs_dev,
            "scale_b": sb_dev,
            "projT": pw_dev,
            "gate_w": g_dev,
            "up_w": u_dev,
            "down_w": d_dev,
        })
    return maps


def _run(inputs, **kw):
    if "nc" not in _CACHE:
        _CACHE["nc"] = _build()
    nc = _CACHE["nc"]
    maps = _prep_inputs(inputs)
    res = run_bass_kernel_spmd(nc, maps, core_ids=list(range(N_CORES)), **kw)
    full = np.concatenate([res.results[g]["outT"] for g in range(N_CORES)], axis=1)
    out = np.ascontiguousarray(full.T)[None].astype(np.float32)
    return out, res


def kernel(**inputs):
    out, _ = _run(inputs)
    return out


def kernel_traced(**inputs):
    out, res = _run(inputs, trace=True)
    return out, res
